# revision 104
# baseline (speedup 1.0000x reference)
"""BitLinear MLP (per-token int8 act fake-quant, per-tensor ternary weight
fake-quant, tanh-gelu) on 8 Trainium2 NeuronCores.

Sharding: data-parallel over tokens (B*S = 16384 -> 2048 tokens/core), weights
replicated. Weights are fake-quantized host-side to ternary fp8e4 plus an fp32
inverse scale.

Key trick: fp8 DoubleRow matmuls at 2x the bf16 rate. The int8-valued
activations xq in [-128,127] are split exactly as xq = hi + lo with
hi = fp8e4_rne(xq) (exactly representable) and lo = xq - hi in [-4,4]
(exactly representable). Two DoubleRow matmul streams (hi and lo), each
processing two 128-deep k-tiles per instruction, reproduce the exact integer
product xq @ wq in fp32 PSUM at twice the bf16 throughput.

Pipeline per core (P=128 token tiles):
  phase 0: load x tile, row absmax -> scale (DVE), round to int via magic-add
           (ACT), hi = fp8(xq) (Pool), lo = xq - hi (DVE), hi/lo bytes
           interleaved so one 2-byte DMA xbar transpose moves both
           -> resident xqT fp8 pairs.
  phase 1: y = xq @ w1q streamed over 1024-col w1 chunks; integer-valued y
           evacuated from PSUM as int16 (|y| < 2^15 whp) to a DRAM scratch;
           evacuations alternate ACT/DVE (all-ACT while the quant pipeline
           loads DVE); running row max of y accumulated per tile.
  tail:    tiles 0-1's gelu/h-quant chains run inside the last w1 chunk's
           matmuls, reusing the idle phase-0 quant buffers plus a dedicated
           hqtA pool in never-overlapped SBUF, so mm2 can start at the
           phase boundary.
  phase 2: per token tile: y16 reloaded as 4 sub-tiles on the ACT dma queue,
           h scale from the phase-1 row max (absmax(gelu row)==gelu(row max)),
           gelu (ACT LUT), magic-round in place (Pool), hi (ACT) / lo (DVE),
           transpose; groups of 4 tiles; within each iq the tile pairs {0,1}
           then {2,3} run 4 resident w2 chunks each, giving late-quantized
           tiles an extra half-iq of runway; w2 chunk loads run on a cursor
           4 chunks ahead of consumption; the next group's quant chains are
           paced proportionally through the group's kc slots (done by slot
           ~21 of 32), with pending transposes force-flushed before any
           matmul that reads them (a read emitted before its write carries
           no dependency).
"""

import sys

sys.path.insert(0, "/opt/trn_rl_repo")

from contextlib import ExitStack

import ml_dtypes
import numpy as np

import concourse.bass as bass
from concourse import bacc
import concourse.mybir as mybir
import concourse.tile as tile
from concourse.alu_op_type import AluOpType as ALU
from concourse.bass_utils import run_bass_kernel_spmd

F32 = mybir.dt.float32
BF16 = mybir.dt.bfloat16
FP8 = mybir.dt.float8e4
I16 = mybir.dt.int16
AXX = mybir.AxisListType.X
GELU = mybir.ActivationFunctionType.Gelu_apprx_tanh
IDENT = mybir.ActivationFunctionType.Identity
DR = mybir.MatmulPerfMode.DoubleRow

B, S, D, H = 4, 4096, 2048, 8192
T = B * S
NCORES = 8
TPC = T // NCORES  # tokens per core
EPS = 1e-5
MAGIC = float(np.float32(1.5 * 2**23))  # add/sub -> round-to-nearest-even
P = 128


CFG = {
    "hqt_bufs": 6,  # tiles 2..15 ring; tiles 0-1 live in hqtA
    "w2_bufs": 6,
    "hst_bufs": 2,
    "hh_bufs": 2,
    "yld_bufs": 4,  # y reload sub-tile ring
    "w1_bufs": 3,
    "yld_nsp": 4,
    "nhq": 4,
    "step_tgt": 21,
    "pair_evac": 0,
    "tail_act_evacs": 0,
    "q01_act_evacs": 1,
    "fill_start": 360,
    "fill_trans": 100,
    "out_bf16": True,
    "pool_mode": "stack",  # tile pool SBUF allocator: stack | queue
    "yld_eng": "scalar",  # engine queue for y16 reloads
    "st_eng": "sync",  # engine queue for y16/out stores
    "tr_eng": "sync",  # engine queue for hqT transposes
}


def build_nc(tpc: int, d: int, h: int) -> bass.Bass:
    NT = tpc // P  # token tiles (16)
    KD = d // P  # layer-1 k-tiles (16)
    KH = h // P  # layer-2 k-tiles (64)
    W1C = 1024  # w1 streamed chunk cols
    NQ = h // W1C  # 8
    TG = 4  # phase-2 token-tile group
    NG = NT // TG
    NI = d // 512  # layer-2 output col chunks (4)
    NKC = CFG.get("NKC", 8)  # w2 k-chunks per iq
    KC = KH // NKC  # k-tiles per w2 chunk
    NHQ = CFG.get("nhq", 4)  # h quantize sub-chunks per tile
    HQC = h // NHQ  # cols per sub-chunk
    KQC = KH // NHQ  # k-tiles per sub-chunk

    nc = bacc.Bacc(trn_type="TRN2")
    x = nc.dram_tensor("x", [tpc, d], F32, kind="ExternalInput")[:]
    w1t = nc.dram_tensor("w1t", [d, h], FP8, kind="ExternalInput")[:]
    w2t = nc.dram_tensor("w2t", [h, d], FP8, kind="ExternalInput")[:]
    wsc = nc.dram_tensor("wsc", [1, 2], F32, kind="ExternalInput")[:]
    out_dt = BF16 if CFG.get("out_bf16") else F32
    out = nc.dram_tensor("out", [tpc, d], out_dt, kind="ExternalOutput")[:]

    def eng(key):
        return getattr(nc, CFG.get(key, "sync"))

    with tile.TileContext(nc, pool_alloc_mode=CFG.get("pool_mode", "stack")) as tc, \
            ExitStack() as ctx:
        const = ctx.enter_context(tc.tile_pool(name="const", bufs=1))
        scl = ctx.enter_context(tc.tile_pool(name="scl", bufs=1))
        dram = ctx.enter_context(tc.tile_pool(name="dram", bufs=1, space="DRAM"))

        wsc_sb = const.tile([P, 2], F32)
        nc.gpsimd.dma_start(out=wsc_sb, in_=wsc.to_broadcast((P, 2)))
        magic_sb = const.tile([P, 1], F32)
        nc.vector.memset(magic_sb, MAGIC)
        nmagic_sb = const.tile([P, 1], F32)
        nc.vector.memset(nmagic_sb, -MAGIC)


        xinv = scl.tile([P, NT], F32)  # (absmax_x/127) * winv1, per tile
        ybuf = dram.tile([tpc, h], I16)
        # running per-tile row max of y, accumulated during phase 1
        ymaxs = [
            scl.tile([P, 1], F32, name=f"ymax{i}", tag="ymax", bufs=NT)
            for i in range(NT)
        ]
        for i in range(NT):
            nc.vector.memset(ymaxs[i], -3.0e38)
        # pools that live across the phase boundary: y reload sub-tiles, the
        # first two h-quant transpose targets, and the small scale tiles, so
        # tiles 0-1's gelu/quant chains can run inside the phase-1 tail
        yld_pool = ctx.enter_context(tc.tile_pool(name="yldq", bufs=CFG["yld_bufs"]))
        hqtA_pool = ctx.enter_context(tc.tile_pool(name="hqtA", bufs=2))
        p2s = ctx.enter_context(tc.tile_pool(name="p2small", bufs=2 * TG))

        ylds: dict = {}  # (tt, j) -> sub tile
        hq_tiles: dict = {}
        pend_tr: list = []
        chp: dict = {}  # chain buffer pools for the current phase

        NSP = CFG.get("yld_nsp", 4)  # y row reloads split into sub-tiles
        YSC = h // NSP

        def ensure_sub(tt, j):
            if (tt, j) in ylds:
                return
            s = yld_pool.tile([P, YSC], I16, tag="yldq", name="yldq")
            eng("yld_eng").dma_start(
                out=s,
                in_=ybuf[tt * P : (tt + 1) * P, j * YSC : (j + 1) * YSC],
            )
            ylds[(tt, j)] = s

        def flush_tr(need=None):
            # transposes go out one step late so their inputs are long since
            # ready and the issuing queue never parks; need=(g, kc) flushes
            # just the entries the imminent matmuls might read
            keep = []
            while pend_tr:
                hh_, tt_, hc_ = pend_tr.pop(0)
                if need is not None:
                    g_, kc_ = need
                    if tt_ >= (g_ + 1) * TG or (
                        tt_ >= g_ * TG and hc_ * KQC > kc_ * KC
                    ):
                        keep.append((hh_, tt_, hc_))
                        continue
                eng("tr_eng").dma_start(
                    out=hq_tiles[tt_][3][:, hc_ * KQC : (hc_ + 1) * KQC, :],
                    in_=hh_.bitcast(BF16).rearrange("p d one -> p (d one)"),
                    transpose=True,
                )
            pend_tr.extend(keep)

        def quant_h_pre(tt):
            # scale chain from the phase-1 running row max of y
            if tt in hq_tiles:
                return
            hm = p2s.tile([P, 1], F32, tag="hm", name="hm")
            nc.scalar.activation(hm, ymaxs[tt], GELU, scale=xinv[:, tt : tt + 1])
            nc.vector.tensor_scalar_max(hm, hm, EPS)
            hs = p2s.tile([P, 1], F32, tag="hs", name="hs")
            nc.vector.reciprocal(hs, hm)
            nc.vector.tensor_scalar(hs, hs, 127.0, None, op0=ALU.mult)
            hinv = p2s.tile([P, 1], F32, tag="hinv", name="hinv")
            nc.vector.tensor_scalar(
                hinv, hm, wsc_sb[:, 1:2], 1.0 / 127.0,
                op0=ALU.mult, op1=ALU.mult,
            )
            if tt < 2:
                hqT16 = hqtA_pool.tile([P, KH, P], BF16, tag="hqTA", name="hqTA")
            else:
                hqT16 = chp["hqt"].tile([P, KH, P], BF16, tag="hqT", name="hqT")
            hq_tiles[tt] = [
                hqT16.bitcast(FP8).rearrange("p k (t two) -> p k t two", two=2),
                hinv,
                hs,
                hqT16,
            ]

        def chain_step(tt, hc, peek=None, hi_pool=False):
            # one NHQ-sub-chunk of the gelu->round->hi/lo->transpose chain
            _, _, hs, hqT16 = hq_tiles[tt]
            ensure_sub(tt, hc * HQC // YSC)
            j = hc * HQC // YSC
            yl = ylds[(tt, j)]
            loc = hc * HQC - j * YSC
            hst_pool, hst_tag = chp["hst"]
            hh_pool, hh_tag = chp["hh"]
            hf = hst_pool.tile([P, HQC], F32, tag=hst_tag, name=hst_tag)
            nc.scalar.activation(
                hf, yl[:, loc : loc + HQC], GELU, scale=xinv[:, tt : tt + 1]
            )
            # round in place: hf <- hf*hs + MAGIC (saves a buffer)
            nc.gpsimd.tensor_scalar(
                hf, hf, hs, MAGIC, op0=ALU.mult, op1=ALU.add
            )
            hh = hh_pool.tile([P, HQC, 2], FP8, tag=hh_tag, name=hh_tag)
            if hi_pool:
                # tail: ACT is loaded with gelu+evacs; cast hi on Pool
                nc.gpsimd.tensor_scalar(
                    hh[:, :, 0], hf, MAGIC, None, op0=ALU.subtract
                )
            else:
                nc.scalar.activation(
                    hh[:, :, 0], hf, IDENT, bias=nmagic_sb[:, 0:1]
                )
            nc.vector.scalar_tensor_tensor(
                hh[:, :, 1], hf, MAGIC, hh[:, :, 0],
                op0=ALU.subtract, op1=ALU.subtract,
            )
            flush_tr()
            pend_tr.append((hh, tt, hc))
            if (hc + 1) * HQC % YSC == 0:
                ylds.pop((tt, j), None)
            if peek is not None and peek[0] in hq_tiles:
                ensure_sub(peek[0], peek[1] * HQC // YSC)

        # ---------- phase 0 + 1: quantize x, y = xq @ w1q -> int16 ----------
        with (
            tc.tile_pool(name="xqt", bufs=1) as xqt_pool,
            tc.tile_pool(name="w1sb", bufs=CFG.get("w1_bufs", 2)) as w1_pool,
            tc.tile_pool(name="xst", bufs=2) as xst,
            tc.tile_pool(name="xqm", bufs=2) as xqm_pool,
            tc.tile_pool(name="xhilo", bufs=2) as xhilo_pool,
            tc.tile_pool(name="p0small", bufs=4) as p0s,
            tc.tile_pool(name="y16", bufs=CFG.get("y16_bufs", 3)) as y16_pool,
            tc.tile_pool(name="mm1", bufs=4, space="PSUM") as mmps,
        ):
            xqT16 = xqt_pool.tile([P, KD, NT * P], BF16)
            xqT8 = xqT16.bitcast(FP8).rearrange("p k (t two) -> p k t two", two=2)

            def quant_x(tt):
                xt = xst.tile([P, d], F32, tag="xt", name="xt")
                if tt < CFG.get("x_split", 0):
                    # startup: halve the load latency by splitting across the
                    # SP and ACT dma queues
                    hd = d // 2
                    nc.sync.dma_start(
                        out=xt[:, :hd], in_=x[tt * P : (tt + 1) * P, :hd]
                    )
                    nc.scalar.dma_start(
                        out=xt[:, hd:], in_=x[tt * P : (tt + 1) * P, hd:]
                    )
                else:
                    nc.sync.dma_start(out=xt, in_=x[tt * P : (tt + 1) * P, :])
                xm = p0s.tile([P, 1], F32, tag="xm", name="xm")
                nc.vector.reduce_max(xm, xt, axis=AXX, apply_absolute_value=True)
                nc.vector.tensor_scalar_max(xm, xm, EPS)
                xs_ = p0s.tile([P, 1], F32, tag="xs", name="xs")
                nc.vector.reciprocal(xs_, xm)
                nc.vector.tensor_scalar(xs_, xs_, 127.0, None, op0=ALU.mult)
                nc.vector.tensor_scalar(
                    xinv[:, tt : tt + 1], xm, wsc_sb[:, 0:1], 1.0 / 127.0,
                    op0=ALU.mult, op1=ALU.mult,
                )
                xq_m = xqm_pool.tile([P, d], F32, tag="xqm", name="xqm")
                nc.scalar.activation(xq_m, xt, IDENT, bias=magic_sb[:, 0:1], scale=xs_)
                hilo = xhilo_pool.tile([P, d, 2], FP8, tag="hilo", name="hilo")
                nc.gpsimd.tensor_scalar(
                    hilo[:, :, 0], xq_m, MAGIC, None, op0=ALU.subtract
                )
                # DVE is the bottleneck engine while the quant pipeline runs
                # (absmax + row-max + evacs); optionally split lo onto Pool
                le = nc.gpsimd if CFG.get("lo_pool") else nc.vector
                le.scalar_tensor_tensor(
                    hilo[:, :, 1], xq_m, MAGIC, hilo[:, :, 0],
                    op0=ALU.subtract, op1=ALU.subtract,
                )
                # first tiles' transposes go via ACT so they don't queue
                # behind the x prefetch loads on SP
                te = nc.scalar if tt < CFG.get("xtr_act", 0) else nc.sync
                te.dma_start(
                    out=xqT16[:, :, tt * P : (tt + 1) * P],
                    in_=hilo.bitcast(BF16).rearrange("p d one -> p (d one)"),
                    transpose=True,
                )

            # keep the PE warm from t~0 so the first real matmuls (at ~23us,
            # after tile 0's quant chain) start at full clock. Source data is
            # tile 15's (not-yet-written, garbage) xqT block -- its transpose
            # lands at ~75us so the WAR ordering is harmless; output goes to
            # a scratch PSUM region reset by the first real start=True.
            # filler source: tile 15's (not-yet-written) xqT block -- its
            # transpose lands at ~75us so the WAR ordering is harmless
            fv = xqT16.bitcast(FP8)
            t15 = NT - 1
            flhs = fv[:, 0:2, t15 * 2 * P : t15 * 2 * P + 128]
            frhs = fv[:, 0:2, t15 * 2 * P : t15 * 2 * P + 256]
            def fill1(n):
                # each burst takes a fresh ring slot so it can never alias a
                # live accumulator
                if not n:
                    return
                dps = mmps.tile([P, W1C], F32, tag="mm", name="mm_fill")
                for _ in range(n):
                    nc.tensor.matmul(
                        dps[:, :256], lhsT=flhs, rhs=frhs, start=True,
                        stop=True, perf_mode=DR, skip_group_check=True,
                    )

            fill1(CFG.get("fill_start", 0))

            for tt in range(4):
                quant_x(tt)

            w1sbs = {}

            def load_w1(q):
                w1sb = w1_pool.tile([P, KD, W1C], FP8, tag="w1sb", name="w1sb")
                for k4 in range(0, KD, 4):
                    nc.sync.dma_start(
                        out=w1sb[:, k4 : k4 + 4, :],
                        in_=w1t[
                            k4 * P : (k4 + 4) * P, q * W1C : (q + 1) * W1C
                        ].rearrange("(kk p) c -> p kk c", p=P),
                    )
                w1sbs[q] = w1sb

            pend_store = []

            def flush_stores():
                while pend_store:
                    y16p, qp, ttp = pend_store.pop(0)
                    eng("st_eng").dma_start(
                        out=ybuf[
                            ttp * P : (ttp + 1) * P, qp * W1C : (qp + 1) * W1C
                        ],
                        in_=y16p,
                    )

            def mm1_block(q, tt):
                w1sb = w1sbs[q]
                ps = mmps.tile([P, W1C], F32, tag="mm", name="mm")
                for kp in range(KD // 2):
                    lhi = xqT8[:, 2 * kp : 2 * kp + 2, tt * P : (tt + 1) * P, 0]
                    llo = xqT8[:, 2 * kp : 2 * kp + 2, tt * P : (tt + 1) * P, 1]
                    for j in range(W1C // 512):
                        rhs = w1sb[:, 2 * kp : 2 * kp + 2, j * 512 : (j + 1) * 512]
                        pj = ps[:, j * 512 : (j + 1) * 512]
                        nc.tensor.matmul(
                            pj, lhsT=lhi, rhs=rhs, start=(kp == 0), stop=False,
                            perf_mode=DR, skip_group_check=True,
                        )
                        nc.tensor.matmul(
                            pj, lhsT=llo, rhs=rhs, start=False,
                            stop=(kp == KD // 2 - 1),
                            perf_mode=DR, skip_group_check=True,
                        )
                y16 = y16_pool.tile([P, W1C], I16, tag="y16", name="y16")
                if q == NQ - 1 and tt >= NT - CFG.get("tail_act_evacs", 2):
                    # the last evacs gate mm2's PSUM reuse: keep them on ACT,
                    # which is free at the very end of the tail, rather than
                    # DVE which still has chain work queued
                    nc.scalar.activation(y16, ps, IDENT)
                elif q < 2 and CFG.get("q01_act_evacs"):
                    # while the quant pipeline runs, DVE is the critical
                    # engine -- keep all evacs on ACT
                    nc.scalar.activation(y16, ps, IDENT)
                elif (q * NT + tt) % 2 == 0:
                    nc.scalar.activation(y16, ps, IDENT)
                else:
                    nc.vector.tensor_scalar(y16, ps, 0.0, None, op0=ALU.add)
                # store is deferred one block so the SP queue never parks on
                # the evacuation's completion
                flush_stores()
                pend_store.append((y16, q, tt))
                cm = p0s.tile([P, 1], F32, tag="cm", name="cm")
                # during the interleaved region DVE is the critical engine
                # (absmax + lo for the quant chain); run the y row-max there
                # on Pool instead
                ce = nc.gpsimd if (q < 2 and CFG.get("cm_pool_q01")) else nc.vector
                ce.reduce_max(cm, y16, axis=AXX)
                ce.tensor_tensor(ymaxs[tt], ymaxs[tt], cm, op=ALU.max)

            # chunks 0 and 1 interleaved per tile: two chunk-blocks of PE work
            # per freshly quantized tile so the PE outruns the quant cadence
            load_w1(0)
            load_w1(1)
            for tt in range(NT):
                if tt + 4 < NT:
                    quant_x(tt + 4)
                if tt == 10:
                    load_w1(2)
                if tt >= 4 and tt % 2 == 0:
                    fill1(CFG.get("fill_q01", 0))
                mm1_block(0, tt)
                mm1_block(1, tt)
            # tiles 0-1's gelu/quant chains run inside the q=NQ-1 tail, using
            # the idle phase-0 quant buffers (same shapes) and hqtA, so the
            # first mm2 group can start right at the phase boundary
            chp["hst"] = (xst, "xt")
            chp["hh"] = (xhilo_pool, "hilo")
            # packed into early tail blocks: the last ~5 blocks stay clean so
            # the final evacs/stores (and the w2 primes queued behind them)
            # aren't pushed past the phase boundary
            tail = {
                1: lambda: (quant_h_pre(0), ensure_sub(0, 0)),
                2: lambda: chain_step(0, 0, peek=(0, 1)),
                4: lambda: chain_step(0, 1, peek=(0, 2)),
                6: lambda: chain_step(0, 2, peek=(0, 3)),
                7: lambda: quant_h_pre(1),
                8: lambda: chain_step(0, 3, peek=(1, 0)),
                10: lambda: chain_step(1, 0, peek=(1, 1)),
                12: lambda: chain_step(1, 1, peek=(1, 2)),
                14: lambda: chain_step(1, 2, peek=(1, 3)),
            }
            for q in range(2, NQ):
                for tt in range(NT):
                    if tt == 2 and q + 1 < NQ:
                        load_w1(q + 1)
                    mm1_block(q, tt)
                    if q == NQ - 1 and tt in tail:
                        tail[tt]()
            flush_stores()

        # ---------- phase 2: gelu, quantize h, out = hq @ w2q ----------
        with (
            tc.tile_pool(name="hqt", bufs=CFG["hqt_bufs"]) as hqt_pool,
            tc.tile_pool(name="w2sb", bufs=CFG["w2_bufs"]) as w2_pool,
            tc.tile_pool(name="hst", bufs=CFG["hst_bufs"]) as hst_pool,
            tc.tile_pool(name="hhilo", bufs=CFG["hh_bufs"]) as hh_pool,
            tc.tile_pool(name="ost", bufs=CFG.get("ost_bufs", 4)) as ost_pool,
            tc.tile_pool(name="mm2", bufs=2 * TG, space="PSUM") as mmps2,
        ):
            chp["hst"] = (hst_pool, "hf")
            chp["hh"] = (hh_pool, "hh")
            chp["hqt"] = hqt_pool
            pend_out = []

            def flush_out():
                while pend_out:
                    otp, ttp, iqp = pend_out.pop(0)
                    eng("st_eng").dma_start(
                        out=out[
                            ttp * P : (ttp + 1) * P, iqp * 512 : (iqp + 1) * 512
                        ],
                        in_=otp,
                    )

            # w2 chunk loads run on a cursor AHEAD of consumption so quant-
            # chain DMA bursts (yld + transposes) never starve the PE of w2
            w2seq = [
                (iq, kc)
                for _g in range(NG)
                for iq in range(NI)
                for kc in range(NKC)
            ]
            w2q: list = []
            w2state = [0]

            def w2_load_one(e=None):
                if w2state[0] >= len(w2seq):
                    return
                iq_, kc_ = w2seq[w2state[0]]
                w2state[0] += 1
                w2sb = w2_pool.tile([P, KC, 512], FP8, tag="w2sb", name="w2sb")
                for k8 in range(0, KC, 8):
                    r0 = (kc_ * KC + k8) * P
                    (e or nc.sync).dma_start(
                        out=w2sb[:, k8 : k8 + 8, :],
                        in_=w2t[
                            r0 : r0 + 8 * P, iq_ * 512 : (iq_ + 1) * 512
                        ].rearrange("(kk p) c -> p kk c", p=P),
                    )
                w2q.append(w2sb)

            # prime w2 BEFORE the transition chains so chunk 0 isn't queued
            # behind their transposes/loads
            W2A = CFG.get("w2_ahead", 4)
            pre_e = getattr(nc, CFG["w2_pre_eng"]) if CFG.get("w2_pre_eng") else None
            for _ in range(W2A):
                w2_load_one(pre_e)

            # PE p-state filler: garbage DoubleRow matmuls on resident hqTA
            # data into a scratch PSUM bank (later reset by a real start=True
            # accumulation). Emitted just before a known PE stall, they keep
            # the tensor engine's ramp warm through the gap so the matmuls
            # after it run at full clock instead of paying the ~3us re-ramp.
            def pe_fill(n, src_tt):
                if not n:
                    return
                fl8 = hq_tiles[src_tt][3].bitcast(FP8).rearrange(
                    "p k t -> p (k t)"
                )
                dps = mmps2.tile([P, 512], F32, tag="mm2", name="ps2_fill")
                lhsT = fl8[:, 0:256].rearrange("p (a b) -> p a b", a=2)
                rhs = fl8[:, 0:1024].rearrange("p (a b) -> p a b", a=2)
                for _ in range(n):
                    nc.tensor.matmul(
                        dps, lhsT=lhsT, rhs=rhs, start=True, stop=True,
                        perf_mode=DR, skip_group_check=True,
                    )

            # transition: only the minimum runs before the first matmuls --
            # tiles 2-3's hc=0 chains (needed by pair23's first kc chunks);
            # the rest interleaves into the g=0 emission stream so the
            # scheduler's coarse engine counters don't gate mm2 on them
            for u in range(2, TG):
                quant_h_pre(u)
            chain_step(1, 3, peek=(2, 0))
            # (2,0)/(3,0) are emitted inside pair01's first kc pass (below)
            # so the first matmuls' queue-tick waits exclude their transposes
            trans_head = [(2, 0), (3, 0)]
            trans_rest = [(2 + (i % 2), 1 + i // 2) for i in range(2 * (NHQ - 1))]

            for g in range(NG):
                # next group's quant chains, paced 2 sub-chunks per 3 kc
                # slots so the last transpose lands before the group ends
                if CFG.get("steps_hc_major"):
                    order = [
                        (u, hc) for hc in range(NHQ) for u in range(TG)
                    ]
                else:
                    order = [
                        (u, hc) for u in range(TG) for hc in range(NHQ)
                    ]
                steps = (
                    [((g + 1) * TG + u, hc) for u, hc in order]
                    if g + 1 < NG
                    else []
                )
                if g == 0:
                    steps = trans_rest + steps
                if g + 1 < NG:
                    for u in range(TG):
                        quant_h_pre((g + 1) * TG + u)
                pe_fill(
                    CFG.get("fill_trans", 0) if g == 0 else CFG.get("fill_grp", 0),
                    g * TG,
                )
                si = 0
                slot = 0
                for iq in range(NI):
                    pss = [
                        mmps2.tile([P, 512], F32, tag="mm2", name=f"ps2_{u}")
                        for u in range(TG)
                    ]

                    def mms(kc, w2sb, u):
                        hqT8 = hq_tiles[g * TG + u][0]
                        for kp in range(KC // 2):
                            kg = kc * KC + 2 * kp
                            lhi = hqT8[:, kg : kg + 2, :, 0]
                            llo = hqT8[:, kg : kg + 2, :, 1]
                            rhs = w2sb[:, 2 * kp : 2 * kp + 2, :]
                            nc.tensor.matmul(
                                pss[u], lhsT=lhi, rhs=rhs,
                                start=(kc == 0 and kp == 0), stop=False,
                                perf_mode=DR, skip_group_check=True,
                            )
                            nc.tensor.matmul(
                                pss[u], lhsT=llo, rhs=rhs, start=False,
                                stop=(kc == NKC - 1 and kp == KC // 2 - 1),
                                perf_mode=DR, skip_group_check=True,
                            )

                    def evac(u):
                        tt = g * TG + u
                        hinv = hq_tiles[tt][1]
                        ot = ost_pool.tile([P, 512], out_dt, tag="ot", name="ot")
                        if u % 2 == 0:
                            nc.scalar.activation(ot, pss[u], IDENT, scale=hinv)
                        else:
                            nc.vector.tensor_scalar(
                                ot, pss[u], hinv, None, op0=ALU.mult
                            )
                        flush_out()
                        pend_out.append((ot, tt, iq))

                    # u-paired: tiles {0,1} run 4 kc chunks, then {2,3} reuse
                    # the same resident w2 chunks -- late-quantized tiles get
                    # an extra half-iq of runway at every group boundary, and
                    # each pair's PSUM banks evacuate as soon as it finishes
                    for kh in range(NKC // 4):
                        chunk4 = [w2q.pop(0) for _ in range(4)]
                        for pi, pair in enumerate(((0, 1), (2, 3))):
                            for i4, w2sb in enumerate(chunk4):
                                # pending transposes a coming matmul reads
                                # MUST be emitted before it (a read emitted
                                # before its write carries no dependency)
                                flush_tr(need=(g, kh * 4 + i4))
                                for u in pair:
                                    mms(kh * 4 + i4, w2sb, u)
                                if g == 0 and iq == 0 and kh == 0 and pi == 0 \
                                        and i4 < len(trans_head):
                                    st = trans_head[i4]
                                    nxt = (
                                        trans_head[i4 + 1]
                                        if i4 + 1 < len(trans_head)
                                        else (steps[0] if steps else None)
                                    )
                                    chain_step(*st, peek=nxt)
                                if pi == 1:
                                    w2_load_one()
                                    # proportional pacing: chain steps spread
                                    # evenly, finishing by slot TGT of 32
                                    TGT = CFG.get("step_tgt", 26)
                                    quota = -(-(slot + 1) * len(steps) // TGT)
                                    while (
                                        si < min(len(steps), quota)
                                        and steps[si][0] in hq_tiles
                                    ):
                                        nxt = (
                                            steps[si + 1]
                                            if si + 1 < len(steps)
                                            else None
                                        )
                                        chain_step(*steps[si], peek=nxt)
                                        si += 1
                                    slot += 1
                            if kh == NKC // 4 - 1 and CFG.get("pair_evac", 1):
                                for u in pair:
                                    evac(u)
                    if not CFG.get("pair_evac", 1):
                        for u in range(TG):
                            evac(u)
                while si < len(steps):
                    quant_h_pre(steps[si][0])
                    chain_step(*steps[si])
                    si += 1
            flush_tr()
            flush_out()
    nc.compile()
    return nc


_wq_cache: dict = {}


def _quant_weight_host(w: np.ndarray):
    """Mirror reference _weight_quant: ternary fp8 values + fp32 inverse scale."""
    import hashlib

    w = np.ascontiguousarray(np.asarray(w, dtype=np.float32))
    key = (w.shape, hashlib.md5(w.view(np.uint8)).hexdigest())
    hit = _wq_cache.get(key)
    if hit is not None:
        return hit
    mean = np.maximum(np.mean(np.abs(w), dtype=np.float32), np.float32(EPS))
    scale = np.float32(1.0) / mean
    tern = np.clip(np.round(w * scale), np.float32(-1.0), np.float32(1.0))
    wT = np.ascontiguousarray(tern.T).astype(ml_dtypes.float8_e4m3)
    winv = np.float32(1.0) / scale
    _wq_cache[key] = (wT, winv)
    return wT, winv


_built: dict = {}


def _get_nc(tpc, d, h):
    key = (tpc, d, h)
    if key not in _built:
        _built[key] = build_nc(*key)
    return _built[key]


def run(inputs, trace=False, shapes=None, ncores=NCORES):
    if shapes is None:
        b, s, d, h = B, S, D, H
    else:
        b, s, d, h = shapes
    t = b * s
    tpc = t // ncores
    x = np.ascontiguousarray(np.asarray(inputs["x"], np.float32).reshape(t, d))
    w1t, winv1 = _quant_weight_host(inputs["w1"])
    w2t, winv2 = _quant_weight_host(inputs["w2"])
    wsc = np.array([[winv1, winv2]], dtype=np.float32)
    in_maps = [
        {
            "x": np.ascontiguousarray(x[c * tpc : (c + 1) * tpc]),
            "w1t": w1t,
            "w2t": w2t,
            "wsc": wsc,
        }
        for c in range(ncores)
    ]
    nc = _get_nc(tpc, d, h)
    res = run_bass_kernel_spmd(
        nc, in_maps, core_ids=list(range(ncores)), trace=False
    )
    outf = np.concatenate(
        [np.asarray(res.results[c]["out"], dtype=np.float32) for c in range(ncores)],
        axis=0,
    )
    return outf.reshape(b, s, d), res


def kernel(**inputs) -> np.ndarray:
    return run(inputs)[0]



# revision 108
# speedup vs baseline: 1.0009x; 1.0009x over previous
"""BitLinear MLP (per-token int8 act fake-quant, per-tensor ternary weight
fake-quant, tanh-gelu) on 8 Trainium2 NeuronCores.

Sharding: data-parallel over tokens (B*S = 16384 -> 2048 tokens/core), weights
replicated. Weights are fake-quantized host-side to ternary fp8e4 plus an fp32
inverse scale.

Key trick: fp8 DoubleRow matmuls at 2x the bf16 rate. The int8-valued
activations xq in [-128,127] are split exactly as xq = hi + lo with
hi = fp8e4_rne(xq) (exactly representable) and lo = xq - hi in [-4,4]
(exactly representable). Two DoubleRow matmul streams (hi and lo), each
processing two 128-deep k-tiles per instruction, reproduce the exact integer
product xq @ wq in fp32 PSUM at twice the bf16 throughput.

Pipeline per core (P=128 token tiles):
  phase 0: load x tile, row absmax -> scale (DVE), round to int via magic-add
           (ACT), hi = fp8(xq) (Pool), lo = xq - hi (DVE), hi/lo bytes
           interleaved so one 2-byte DMA xbar transpose moves both
           -> resident xqT fp8 pairs.
  phase 1: y = xq @ w1q streamed over 1024-col w1 chunks; integer-valued y
           evacuated from PSUM as int16 (|y| < 2^15 whp) to a DRAM scratch;
           evacuations alternate ACT/DVE (all-ACT while the quant pipeline
           loads DVE); running row max of y accumulated per tile.
  tail:    tiles 0-1's gelu/h-quant chains run inside the last w1 chunk's
           matmuls, reusing the idle phase-0 quant buffers plus a dedicated
           hqtA pool in never-overlapped SBUF, so mm2 can start at the
           phase boundary.
  phase 2: per token tile: y16 reloaded as 4 sub-tiles on the ACT dma queue,
           h scale from the phase-1 row max (absmax(gelu row)==gelu(row max)),
           gelu (ACT LUT), magic-round in place (Pool), hi (ACT) / lo (DVE),
           transpose; groups of 4 tiles; within each iq the tile pairs {0,1}
           then {2,3} run 4 resident w2 chunks each, giving late-quantized
           tiles an extra half-iq of runway; w2 chunk loads run on a cursor
           4 chunks ahead of consumption; the next group's quant chains are
           paced proportionally through the group's kc slots (done by slot
           ~21 of 32), with pending transposes force-flushed before any
           matmul that reads them (a read emitted before its write carries
           no dependency).
"""

import sys

sys.path.insert(0, "/opt/trn_rl_repo")

from contextlib import ExitStack

import ml_dtypes
import numpy as np

import concourse.bass as bass
from concourse import bacc
import concourse.mybir as mybir
import concourse.tile as tile
from concourse.alu_op_type import AluOpType as ALU
from concourse.bass_utils import run_bass_kernel_spmd

F32 = mybir.dt.float32
BF16 = mybir.dt.bfloat16
FP8 = mybir.dt.float8e4
I16 = mybir.dt.int16
AXX = mybir.AxisListType.X
GELU = mybir.ActivationFunctionType.Gelu_apprx_tanh
IDENT = mybir.ActivationFunctionType.Identity
DR = mybir.MatmulPerfMode.DoubleRow

B, S, D, H = 4, 4096, 2048, 8192
T = B * S
NCORES = 8
TPC = T // NCORES  # tokens per core
EPS = 1e-5
MAGIC = float(np.float32(1.5 * 2**23))  # add/sub -> round-to-nearest-even
P = 128


CFG = {
    "hqt_bufs": 6,  # tiles 2..15 ring; tiles 0-1 live in hqtA
    "w2_bufs": 6,
    "hst_bufs": 2,
    "hh_bufs": 2,
    "yld_bufs": 4,  # y reload sub-tile ring
    "w1_bufs": 3,
    "yld_nsp": 4,
    "nhq": 4,
    "step_tgt": 21,
    "pair_evac": 0,
    "tail_act_evacs": 0,
    "q01_act_evacs": 1,
    "fill_start": 360,
    "fill_trans": 100,
    "evac_dve": 1,
    "out_bf16": True,
    "pool_mode": "stack",  # tile pool SBUF allocator: stack | queue
    "yld_eng": "scalar",  # engine queue for y16 reloads
    "st_eng": "sync",  # engine queue for y16/out stores
    "tr_eng": "sync",  # engine queue for hqT transposes
}


def build_nc(tpc: int, d: int, h: int) -> bass.Bass:
    NT = tpc // P  # token tiles (16)
    KD = d // P  # layer-1 k-tiles (16)
    KH = h // P  # layer-2 k-tiles (64)
    W1C = 1024  # w1 streamed chunk cols
    NQ = h // W1C  # 8
    TG = 4  # phase-2 token-tile group
    NG = NT // TG
    NI = d // 512  # layer-2 output col chunks (4)
    NKC = CFG.get("NKC", 8)  # w2 k-chunks per iq
    KC = KH // NKC  # k-tiles per w2 chunk
    NHQ = CFG.get("nhq", 4)  # h quantize sub-chunks per tile
    HQC = h // NHQ  # cols per sub-chunk
    KQC = KH // NHQ  # k-tiles per sub-chunk

    nc = bacc.Bacc(trn_type="TRN2")
    x = nc.dram_tensor("x", [tpc, d], F32, kind="ExternalInput")[:]
    w1t = nc.dram_tensor("w1t", [d, h], FP8, kind="ExternalInput")[:]
    w2t = nc.dram_tensor("w2t", [h, d], FP8, kind="ExternalInput")[:]
    wsc = nc.dram_tensor("wsc", [1, 2], F32, kind="ExternalInput")[:]
    out_dt = BF16 if CFG.get("out_bf16") else F32
    out = nc.dram_tensor("out", [tpc, d], out_dt, kind="ExternalOutput")[:]

    def eng(key):
        return getattr(nc, CFG.get(key, "sync"))

    with tile.TileContext(nc, pool_alloc_mode=CFG.get("pool_mode", "stack")) as tc, \
            ExitStack() as ctx:
        const = ctx.enter_context(tc.tile_pool(name="const", bufs=1))
        scl = ctx.enter_context(tc.tile_pool(name="scl", bufs=1))
        dram = ctx.enter_context(tc.tile_pool(name="dram", bufs=1, space="DRAM"))

        wsc_sb = const.tile([P, 2], F32)
        nc.gpsimd.dma_start(out=wsc_sb, in_=wsc.to_broadcast((P, 2)))
        magic_sb = const.tile([P, 1], F32)
        nc.vector.memset(magic_sb, MAGIC)
        nmagic_sb = const.tile([P, 1], F32)
        nc.vector.memset(nmagic_sb, -MAGIC)


        xinv = scl.tile([P, NT], F32)  # (absmax_x/127) * winv1, per tile
        ybuf = dram.tile([tpc, h], I16)
        # running per-tile row max of y, accumulated during phase 1
        ymaxs = [
            scl.tile([P, 1], F32, name=f"ymax{i}", tag="ymax", bufs=NT)
            for i in range(NT)
        ]
        for i in range(NT):
            nc.vector.memset(ymaxs[i], -3.0e38)
        # pools that live across the phase boundary: y reload sub-tiles, the
        # first two h-quant transpose targets, and the small scale tiles, so
        # tiles 0-1's gelu/quant chains can run inside the phase-1 tail
        yld_pool = ctx.enter_context(tc.tile_pool(name="yldq", bufs=CFG["yld_bufs"]))
        hqtA_pool = ctx.enter_context(tc.tile_pool(name="hqtA", bufs=2))
        p2s = ctx.enter_context(tc.tile_pool(name="p2small", bufs=2 * TG))

        ylds: dict = {}  # (tt, j) -> sub tile
        hq_tiles: dict = {}
        pend_tr: list = []
        chp: dict = {}  # chain buffer pools for the current phase

        NSP = CFG.get("yld_nsp", 4)  # y row reloads split into sub-tiles
        YSC = h // NSP

        def ensure_sub(tt, j):
            if (tt, j) in ylds:
                return
            s = yld_pool.tile([P, YSC], I16, tag="yldq", name="yldq")
            eng("yld_eng").dma_start(
                out=s,
                in_=ybuf[tt * P : (tt + 1) * P, j * YSC : (j + 1) * YSC],
            )
            ylds[(tt, j)] = s

        def flush_tr(need=None):
            # transposes go out one step late so their inputs are long since
            # ready and the issuing queue never parks; need=(g, kc) flushes
            # just the entries the imminent matmuls might read
            keep = []
            while pend_tr:
                hh_, tt_, hc_ = pend_tr.pop(0)
                if need is not None:
                    g_, kc_ = need
                    if tt_ >= (g_ + 1) * TG or (
                        tt_ >= g_ * TG and hc_ * KQC > kc_ * KC
                    ):
                        keep.append((hh_, tt_, hc_))
                        continue
                eng("tr_eng").dma_start(
                    out=hq_tiles[tt_][3][:, hc_ * KQC : (hc_ + 1) * KQC, :],
                    in_=hh_.bitcast(BF16).rearrange("p d one -> p (d one)"),
                    transpose=True,
                )
            pend_tr.extend(keep)

        def quant_h_pre(tt):
            # scale chain from the phase-1 running row max of y
            if tt in hq_tiles:
                return
            hm = p2s.tile([P, 1], F32, tag="hm", name="hm")
            nc.scalar.activation(hm, ymaxs[tt], GELU, scale=xinv[:, tt : tt + 1])
            nc.vector.tensor_scalar_max(hm, hm, EPS)
            hs = p2s.tile([P, 1], F32, tag="hs", name="hs")
            nc.vector.reciprocal(hs, hm)
            nc.vector.tensor_scalar(hs, hs, 127.0, None, op0=ALU.mult)
            hinv = p2s.tile([P, 1], F32, tag="hinv", name="hinv")
            nc.vector.tensor_scalar(
                hinv, hm, wsc_sb[:, 1:2], 1.0 / 127.0,
                op0=ALU.mult, op1=ALU.mult,
            )
            if tt < 2:
                hqT16 = hqtA_pool.tile([P, KH, P], BF16, tag="hqTA", name="hqTA")
            else:
                hqT16 = chp["hqt"].tile([P, KH, P], BF16, tag="hqT", name="hqT")
            hq_tiles[tt] = [
                hqT16.bitcast(FP8).rearrange("p k (t two) -> p k t two", two=2),
                hinv,
                hs,
                hqT16,
            ]

        def chain_step(tt, hc, peek=None, hi_pool=False):
            # one NHQ-sub-chunk of the gelu->round->hi/lo->transpose chain
            _, _, hs, hqT16 = hq_tiles[tt]
            ensure_sub(tt, hc * HQC // YSC)
            j = hc * HQC // YSC
            yl = ylds[(tt, j)]
            loc = hc * HQC - j * YSC
            hst_pool, hst_tag = chp["hst"]
            hh_pool, hh_tag = chp["hh"]
            hf = hst_pool.tile([P, HQC], F32, tag=hst_tag, name=hst_tag)
            nc.scalar.activation(
                hf, yl[:, loc : loc + HQC], GELU, scale=xinv[:, tt : tt + 1]
            )
            # round in place: hf <- hf*hs + MAGIC (saves a buffer)
            nc.gpsimd.tensor_scalar(
                hf, hf, hs, MAGIC, op0=ALU.mult, op1=ALU.add
            )
            hh = hh_pool.tile([P, HQC, 2], FP8, tag=hh_tag, name=hh_tag)
            if hi_pool:
                # tail: ACT is loaded with gelu+evacs; cast hi on Pool
                nc.gpsimd.tensor_scalar(
                    hh[:, :, 0], hf, MAGIC, None, op0=ALU.subtract
                )
            else:
                nc.scalar.activation(
                    hh[:, :, 0], hf, IDENT, bias=nmagic_sb[:, 0:1]
                )
            nc.vector.scalar_tensor_tensor(
                hh[:, :, 1], hf, MAGIC, hh[:, :, 0],
                op0=ALU.subtract, op1=ALU.subtract,
            )
            flush_tr()
            pend_tr.append((hh, tt, hc))
            if (hc + 1) * HQC % YSC == 0:
                ylds.pop((tt, j), None)
            if peek is not None and peek[0] in hq_tiles:
                ensure_sub(peek[0], peek[1] * HQC // YSC)

        # ---------- phase 0 + 1: quantize x, y = xq @ w1q -> int16 ----------
        with (
            tc.tile_pool(name="xqt", bufs=1) as xqt_pool,
            tc.tile_pool(name="w1sb", bufs=CFG.get("w1_bufs", 2)) as w1_pool,
            tc.tile_pool(name="xst", bufs=2) as xst,
            tc.tile_pool(name="xqm", bufs=2) as xqm_pool,
            tc.tile_pool(name="xhilo", bufs=2) as xhilo_pool,
            tc.tile_pool(name="p0small", bufs=4) as p0s,
            tc.tile_pool(name="y16", bufs=CFG.get("y16_bufs", 3)) as y16_pool,
            tc.tile_pool(name="mm1", bufs=4, space="PSUM") as mmps,
        ):
            xqT16 = xqt_pool.tile([P, KD, NT * P], BF16)
            xqT8 = xqT16.bitcast(FP8).rearrange("p k (t two) -> p k t two", two=2)

            def quant_x(tt):
                xt = xst.tile([P, d], F32, tag="xt", name="xt")
                if tt < CFG.get("x_split", 0):
                    # startup: halve the load latency by splitting across the
                    # SP and ACT dma queues
                    hd = d // 2
                    nc.sync.dma_start(
                        out=xt[:, :hd], in_=x[tt * P : (tt + 1) * P, :hd]
                    )
                    nc.scalar.dma_start(
                        out=xt[:, hd:], in_=x[tt * P : (tt + 1) * P, hd:]
                    )
                else:
                    nc.sync.dma_start(out=xt, in_=x[tt * P : (tt + 1) * P, :])
                xm = p0s.tile([P, 1], F32, tag="xm", name="xm")
                nc.vector.reduce_max(xm, xt, axis=AXX, apply_absolute_value=True)
                nc.vector.tensor_scalar_max(xm, xm, EPS)
                xs_ = p0s.tile([P, 1], F32, tag="xs", name="xs")
                nc.vector.reciprocal(xs_, xm)
                nc.vector.tensor_scalar(xs_, xs_, 127.0, None, op0=ALU.mult)
                nc.vector.tensor_scalar(
                    xinv[:, tt : tt + 1], xm, wsc_sb[:, 0:1], 1.0 / 127.0,
                    op0=ALU.mult, op1=ALU.mult,
                )
                xq_m = xqm_pool.tile([P, d], F32, tag="xqm", name="xqm")
                nc.scalar.activation(xq_m, xt, IDENT, bias=magic_sb[:, 0:1], scale=xs_)
                hilo = xhilo_pool.tile([P, d, 2], FP8, tag="hilo", name="hilo")
                nc.gpsimd.tensor_scalar(
                    hilo[:, :, 0], xq_m, MAGIC, None, op0=ALU.subtract
                )
                # DVE is the bottleneck engine while the quant pipeline runs
                # (absmax + row-max + evacs); optionally split lo onto Pool
                le = nc.gpsimd if CFG.get("lo_pool") else nc.vector
                le.scalar_tensor_tensor(
                    hilo[:, :, 1], xq_m, MAGIC, hilo[:, :, 0],
                    op0=ALU.subtract, op1=ALU.subtract,
                )
                # first tiles' transposes go via ACT so they don't queue
                # behind the x prefetch loads on SP
                te = nc.scalar if tt < CFG.get("xtr_act", 0) else nc.sync
                te.dma_start(
                    out=xqT16[:, :, tt * P : (tt + 1) * P],
                    in_=hilo.bitcast(BF16).rearrange("p d one -> p (d one)"),
                    transpose=True,
                )

            # keep the PE warm from t~0 so the first real matmuls (at ~23us,
            # after tile 0's quant chain) start at full clock. Source data is
            # tile 15's (not-yet-written, garbage) xqT block -- its transpose
            # lands at ~75us so the WAR ordering is harmless; output goes to
            # a scratch PSUM region reset by the first real start=True.
            # filler source: tile 15's (not-yet-written) xqT block -- its
            # transpose lands at ~75us so the WAR ordering is harmless
            fv = xqT16.bitcast(FP8)
            t15 = NT - 1
            flhs = fv[:, 0:2, t15 * 2 * P : t15 * 2 * P + 128]
            frhs = fv[:, 0:2, t15 * 2 * P : t15 * 2 * P + 256]
            def fill1(n):
                # each burst takes a fresh ring slot so it can never alias a
                # live accumulator
                if not n:
                    return
                dps = mmps.tile([P, W1C], F32, tag="mm", name="mm_fill")
                for _ in range(n):
                    nc.tensor.matmul(
                        dps[:, :256], lhsT=flhs, rhs=frhs, start=True,
                        stop=True, perf_mode=DR, skip_group_check=True,
                    )

            fill1(CFG.get("fill_start", 0))

            for tt in range(4):
                quant_x(tt)

            w1sbs = {}

            def load_w1(q):
                w1sb = w1_pool.tile([P, KD, W1C], FP8, tag="w1sb", name="w1sb")
                for k4 in range(0, KD, 4):
                    nc.sync.dma_start(
                        out=w1sb[:, k4 : k4 + 4, :],
                        in_=w1t[
                            k4 * P : (k4 + 4) * P, q * W1C : (q + 1) * W1C
                        ].rearrange("(kk p) c -> p kk c", p=P),
                    )
                w1sbs[q] = w1sb

            pend_store = []

            def flush_stores():
                while pend_store:
                    y16p, qp, ttp = pend_store.pop(0)
                    eng("st_eng").dma_start(
                        out=ybuf[
                            ttp * P : (ttp + 1) * P, qp * W1C : (qp + 1) * W1C
                        ],
                        in_=y16p,
                    )

            def mm1_block(q, tt):
                w1sb = w1sbs[q]
                ps = mmps.tile([P, W1C], F32, tag="mm", name="mm")
                for kp in range(KD // 2):
                    lhi = xqT8[:, 2 * kp : 2 * kp + 2, tt * P : (tt + 1) * P, 0]
                    llo = xqT8[:, 2 * kp : 2 * kp + 2, tt * P : (tt + 1) * P, 1]
                    for j in range(W1C // 512):
                        rhs = w1sb[:, 2 * kp : 2 * kp + 2, j * 512 : (j + 1) * 512]
                        pj = ps[:, j * 512 : (j + 1) * 512]
                        nc.tensor.matmul(
                            pj, lhsT=lhi, rhs=rhs, start=(kp == 0), stop=False,
                            perf_mode=DR, skip_group_check=True,
                        )
                        nc.tensor.matmul(
                            pj, lhsT=llo, rhs=rhs, start=False,
                            stop=(kp == KD // 2 - 1),
                            perf_mode=DR, skip_group_check=True,
                        )
                y16 = y16_pool.tile([P, W1C], I16, tag="y16", name="y16")
                if q == NQ - 1 and tt >= NT - CFG.get("tail_act_evacs", 2):
                    # the last evacs gate mm2's PSUM reuse: keep them on ACT,
                    # which is free at the very end of the tail, rather than
                    # DVE which still has chain work queued
                    nc.scalar.activation(y16, ps, IDENT)
                elif q < 2 and CFG.get("q01_act_evacs"):
                    # while the quant pipeline runs, DVE is the critical
                    # engine -- keep all evacs on ACT
                    nc.scalar.activation(y16, ps, IDENT)
                elif (q * NT + tt) % 2 == 0:
                    nc.scalar.activation(y16, ps, IDENT)
                else:
                    nc.vector.tensor_scalar(y16, ps, 0.0, None, op0=ALU.add)
                # store is deferred one block so the SP queue never parks on
                # the evacuation's completion
                flush_stores()
                pend_store.append((y16, q, tt))
                cm = p0s.tile([P, 1], F32, tag="cm", name="cm")
                # during the interleaved region DVE is the critical engine
                # (absmax + lo for the quant chain); run the y row-max there
                # on Pool instead
                ce = nc.gpsimd if (q < 2 and CFG.get("cm_pool_q01")) else nc.vector
                ce.reduce_max(cm, y16, axis=AXX)
                ce.tensor_tensor(ymaxs[tt], ymaxs[tt], cm, op=ALU.max)

            # chunks 0 and 1 interleaved per tile: two chunk-blocks of PE work
            # per freshly quantized tile so the PE outruns the quant cadence
            load_w1(0)
            load_w1(1)
            for tt in range(NT):
                if tt + 4 < NT:
                    quant_x(tt + 4)
                if tt == 10:
                    load_w1(2)
                if tt >= 4 and tt % 2 == 0:
                    fill1(CFG.get("fill_q01", 0))
                mm1_block(0, tt)
                mm1_block(1, tt)
            # tiles 0-1's gelu/quant chains run inside the q=NQ-1 tail, using
            # the idle phase-0 quant buffers (same shapes) and hqtA, so the
            # first mm2 group can start right at the phase boundary
            chp["hst"] = (xst, "xt")
            chp["hh"] = (xhilo_pool, "hilo")
            # packed into early tail blocks: the last ~5 blocks stay clean so
            # the final evacs/stores (and the w2 primes queued behind them)
            # aren't pushed past the phase boundary
            tail = {
                1: lambda: (quant_h_pre(0), ensure_sub(0, 0)),
                2: lambda: chain_step(0, 0, peek=(0, 1)),
                4: lambda: chain_step(0, 1, peek=(0, 2)),
                6: lambda: chain_step(0, 2, peek=(0, 3)),
                7: lambda: quant_h_pre(1),
                8: lambda: chain_step(0, 3, peek=(1, 0)),
                10: lambda: chain_step(1, 0, peek=(1, 1)),
                12: lambda: chain_step(1, 1, peek=(1, 2)),
                14: lambda: chain_step(1, 2, peek=(1, 3)),
            }
            for q in range(2, NQ):
                for tt in range(NT):
                    if tt == 2 and q + 1 < NQ:
                        load_w1(q + 1)
                    mm1_block(q, tt)
                    if q == NQ - 1 and tt in tail:
                        tail[tt]()
            flush_stores()

        # ---------- phase 2: gelu, quantize h, out = hq @ w2q ----------
        with (
            tc.tile_pool(name="hqt", bufs=CFG["hqt_bufs"]) as hqt_pool,
            tc.tile_pool(name="w2sb", bufs=CFG["w2_bufs"]) as w2_pool,
            tc.tile_pool(name="hst", bufs=CFG["hst_bufs"]) as hst_pool,
            tc.tile_pool(name="hhilo", bufs=CFG["hh_bufs"]) as hh_pool,
            tc.tile_pool(name="ost", bufs=CFG.get("ost_bufs", 4)) as ost_pool,
            tc.tile_pool(name="mm2", bufs=2 * TG, space="PSUM") as mmps2,
        ):
            chp["hst"] = (hst_pool, "hf")
            chp["hh"] = (hh_pool, "hh")
            chp["hqt"] = hqt_pool
            pend_out = []

            def flush_out():
                while pend_out:
                    otp, ttp, iqp = pend_out.pop(0)
                    eng("st_eng").dma_start(
                        out=out[
                            ttp * P : (ttp + 1) * P, iqp * 512 : (iqp + 1) * 512
                        ],
                        in_=otp,
                    )

            # w2 chunk loads run on a cursor AHEAD of consumption so quant-
            # chain DMA bursts (yld + transposes) never starve the PE of w2
            w2seq = [
                (iq, kc)
                for _g in range(NG)
                for iq in range(NI)
                for kc in range(NKC)
            ]
            w2q: list = []
            w2state = [0]

            def w2_load_one(e=None):
                if w2state[0] >= len(w2seq):
                    return
                iq_, kc_ = w2seq[w2state[0]]
                w2state[0] += 1
                w2sb = w2_pool.tile([P, KC, 512], FP8, tag="w2sb", name="w2sb")
                w2ls = CFG.get("w2_ldsplit", 8)
                for k8 in range(0, KC, w2ls):
                    r0 = (kc_ * KC + k8) * P
                    (e or nc.sync).dma_start(
                        out=w2sb[:, k8 : k8 + w2ls, :],
                        in_=w2t[
                            r0 : r0 + w2ls * P, iq_ * 512 : (iq_ + 1) * 512
                        ].rearrange("(kk p) c -> p kk c", p=P),
                    )
                w2q.append(w2sb)

            # prime w2 BEFORE the transition chains so chunk 0 isn't queued
            # behind their transposes/loads
            W2A = CFG.get("w2_ahead", 4)
            pre_e = getattr(nc, CFG["w2_pre_eng"]) if CFG.get("w2_pre_eng") else None
            for _ in range(W2A):
                w2_load_one(pre_e)

            # PE p-state filler: garbage DoubleRow matmuls on resident hqTA
            # data into a scratch PSUM bank (later reset by a real start=True
            # accumulation). Emitted just before a known PE stall, they keep
            # the tensor engine's ramp warm through the gap so the matmuls
            # after it run at full clock instead of paying the ~3us re-ramp.
            def pe_fill(n, src_tt):
                if not n:
                    return
                fl8 = hq_tiles[src_tt][3].bitcast(FP8).rearrange(
                    "p k t -> p (k t)"
                )
                dps = mmps2.tile([P, 512], F32, tag="mm2", name="ps2_fill")
                lhsT = fl8[:, 0:256].rearrange("p (a b) -> p a b", a=2)
                rhs = fl8[:, 0:1024].rearrange("p (a b) -> p a b", a=2)
                for _ in range(n):
                    nc.tensor.matmul(
                        dps, lhsT=lhsT, rhs=rhs, start=True, stop=True,
                        perf_mode=DR, skip_group_check=True,
                    )

            # transition: only the minimum runs before the first matmuls --
            # tiles 2-3's hc=0 chains (needed by pair23's first kc chunks);
            # the rest interleaves into the g=0 emission stream so the
            # scheduler's coarse engine counters don't gate mm2 on them
            for u in range(2, TG):
                quant_h_pre(u)
            chain_step(1, 3, peek=(2, 0))
            # (2,0)/(3,0) are emitted inside pair01's first kc pass (below)
            # so the first matmuls' queue-tick waits exclude their transposes
            trans_head = [(2, 0), (3, 0)]
            trans_rest = [(2 + (i % 2), 1 + i // 2) for i in range(2 * (NHQ - 1))]

            for g in range(NG):
                # next group's quant chains, paced 2 sub-chunks per 3 kc
                # slots so the last transpose lands before the group ends
                if CFG.get("steps_hc_major"):
                    order = [
                        (u, hc) for hc in range(NHQ) for u in range(TG)
                    ]
                else:
                    order = [
                        (u, hc) for u in range(TG) for hc in range(NHQ)
                    ]
                steps = (
                    [((g + 1) * TG + u, hc) for u, hc in order]
                    if g + 1 < NG
                    else []
                )
                if g == 0:
                    steps = trans_rest + steps
                if g + 1 < NG:
                    for u in range(TG):
                        quant_h_pre((g + 1) * TG + u)
                pe_fill(
                    CFG.get("fill_trans", 0) if g == 0 else CFG.get("fill_grp", 0),
                    g * TG,
                )
                si = 0
                slot = 0
                for iq in range(NI):
                    pss = [
                        mmps2.tile([P, 512], F32, tag="mm2", name=f"ps2_{u}")
                        for u in range(TG)
                    ]

                    def mms(kc, w2sb, u):
                        hqT8 = hq_tiles[g * TG + u][0]
                        for kp in range(KC // 2):
                            kg = kc * KC + 2 * kp
                            lhi = hqT8[:, kg : kg + 2, :, 0]
                            llo = hqT8[:, kg : kg + 2, :, 1]
                            rhs = w2sb[:, 2 * kp : 2 * kp + 2, :]
                            nc.tensor.matmul(
                                pss[u], lhsT=lhi, rhs=rhs,
                                start=(kc == 0 and kp == 0), stop=False,
                                perf_mode=DR, skip_group_check=True,
                            )
                            nc.tensor.matmul(
                                pss[u], lhsT=llo, rhs=rhs, start=False,
                                stop=(kc == NKC - 1 and kp == KC // 2 - 1),
                                perf_mode=DR, skip_group_check=True,
                            )

                    def evac(u):
                        tt = g * TG + u
                        hinv = hq_tiles[tt][1]
                        ot = ost_pool.tile([P, 512], out_dt, tag="ot", name="ot")
                        if u % 2 == 0 and not CFG.get("evac_dve"):
                            nc.scalar.activation(ot, pss[u], IDENT, scale=hinv)
                        else:
                            nc.vector.tensor_scalar(
                                ot, pss[u], hinv, None, op0=ALU.mult
                            )
                        flush_out()
                        pend_out.append((ot, tt, iq))

                    # u-paired: tiles {0,1} run 4 kc chunks, then {2,3} reuse
                    # the same resident w2 chunks -- late-quantized tiles get
                    # an extra half-iq of runway at every group boundary, and
                    # each pair's PSUM banks evacuate as soon as it finishes
                    for kh in range(NKC // 4):
                        chunk4 = [w2q.pop(0) for _ in range(4)]
                        for pi, pair in enumerate(((0, 1), (2, 3))):
                            for i4, w2sb in enumerate(chunk4):
                                # pending transposes a coming matmul reads
                                # MUST be emitted before it (a read emitted
                                # before its write carries no dependency)
                                flush_tr(need=(g, kh * 4 + i4))
                                for u in pair:
                                    mms(kh * 4 + i4, w2sb, u)
                                if g == 0 and iq == 0 and kh == 0 and pi == 0 \
                                        and i4 < len(trans_head):
                                    st = trans_head[i4]
                                    nxt = (
                                        trans_head[i4 + 1]
                                        if i4 + 1 < len(trans_head)
                                        else (steps[0] if steps else None)
                                    )
                                    chain_step(*st, peek=nxt)
                                if pi == 1:
                                    w2_load_one()
                                    # proportional pacing: chain steps spread
                                    # evenly, finishing by slot TGT of 32
                                    TGT = CFG.get("step_tgt", 26)
                                    quota = -(-(slot + 1) * len(steps) // TGT)
                                    while (
                                        si < min(len(steps), quota)
                                        and steps[si][0] in hq_tiles
                                    ):
                                        nxt = (
                                            steps[si + 1]
                                            if si + 1 < len(steps)
                                            else None
                                        )
                                        chain_step(*steps[si], peek=nxt)
                                        si += 1
                                    slot += 1
                            if kh == NKC // 4 - 1 and CFG.get("pair_evac", 1):
                                for u in pair:
                                    evac(u)
                    if not CFG.get("pair_evac", 1):
                        for u in range(TG):
                            evac(u)
                while si < len(steps):
                    quant_h_pre(steps[si][0])
                    chain_step(*steps[si])
                    si += 1
            flush_tr()
            flush_out()
    nc.compile()
    return nc


_wq_cache: dict = {}


def _quant_weight_host(w: np.ndarray):
    """Mirror reference _weight_quant: ternary fp8 values + fp32 inverse scale."""
    import hashlib

    w = np.ascontiguousarray(np.asarray(w, dtype=np.float32))
    key = (w.shape, hashlib.md5(w.view(np.uint8)).hexdigest())
    hit = _wq_cache.get(key)
    if hit is not None:
        return hit
    mean = np.maximum(np.mean(np.abs(w), dtype=np.float32), np.float32(EPS))
    scale = np.float32(1.0) / mean
    tern = np.clip(np.round(w * scale), np.float32(-1.0), np.float32(1.0))
    wT = np.ascontiguousarray(tern.T).astype(ml_dtypes.float8_e4m3)
    winv = np.float32(1.0) / scale
    _wq_cache[key] = (wT, winv)
    return wT, winv


_built: dict = {}


def _get_nc(tpc, d, h):
    key = (tpc, d, h)
    if key not in _built:
        _built[key] = build_nc(*key)
    return _built[key]


def run(inputs, trace=False, shapes=None, ncores=NCORES):
    if shapes is None:
        b, s, d, h = B, S, D, H
    else:
        b, s, d, h = shapes
    t = b * s
    tpc = t // ncores
    x = np.ascontiguousarray(np.asarray(inputs["x"], np.float32).reshape(t, d))
    w1t, winv1 = _quant_weight_host(inputs["w1"])
    w2t, winv2 = _quant_weight_host(inputs["w2"])
    wsc = np.array([[winv1, winv2]], dtype=np.float32)
    in_maps = [
        {
            "x": np.ascontiguousarray(x[c * tpc : (c + 1) * tpc]),
            "w1t": w1t,
            "w2t": w2t,
            "wsc": wsc,
        }
        for c in range(ncores)
    ]
    nc = _get_nc(tpc, d, h)
    res = run_bass_kernel_spmd(
        nc, in_maps, core_ids=list(range(ncores)), trace=False
    )
    outf = np.concatenate(
        [np.asarray(res.results[c]["out"], dtype=np.float32) for c in range(ncores)],
        axis=0,
    )
    return outf.reshape(b, s, d), res


def kernel(**inputs) -> np.ndarray:
    return run(inputs)[0]



# revision 109
# speedup vs baseline: 1.0062x; 1.0052x over previous
"""BitLinear MLP (per-token int8 act fake-quant, per-tensor ternary weight
fake-quant, tanh-gelu) on 8 Trainium2 NeuronCores.

Sharding: data-parallel over tokens (B*S = 16384 -> 2048 tokens/core), weights
replicated. Weights are fake-quantized host-side to ternary fp8e4 plus an fp32
inverse scale.

Key trick: fp8 DoubleRow matmuls at 2x the bf16 rate. The int8-valued
activations xq in [-128,127] are split exactly as xq = hi + lo with
hi = fp8e4_rne(xq) (exactly representable) and lo = xq - hi in [-4,4]
(exactly representable). Two DoubleRow matmul streams (hi and lo), each
processing two 128-deep k-tiles per instruction, reproduce the exact integer
product xq @ wq in fp32 PSUM at twice the bf16 throughput.

Pipeline per core (P=128 token tiles):
  phase 0: load x tile, row absmax -> scale (DVE), round to int via magic-add
           (ACT), hi = fp8(xq) (Pool), lo = xq - hi (DVE), hi/lo bytes
           interleaved so one 2-byte DMA xbar transpose moves both
           -> resident xqT fp8 pairs.
  phase 1: y = xq @ w1q streamed over 1024-col w1 chunks; integer-valued y
           evacuated from PSUM as int16 (|y| < 2^15 whp) to a DRAM scratch;
           evacuations alternate ACT/DVE (all-ACT while the quant pipeline
           loads DVE); running row max of y accumulated per tile.
  tail:    tiles 0-1's gelu/h-quant chains run inside the last w1 chunk's
           matmuls, reusing the idle phase-0 quant buffers plus a dedicated
           hqtA pool in never-overlapped SBUF, so mm2 can start at the
           phase boundary.
  phase 2: per token tile: y16 reloaded as 4 sub-tiles on the ACT dma queue,
           h scale from the phase-1 row max (absmax(gelu row)==gelu(row max)),
           gelu (ACT LUT), magic-round in place (Pool), hi (ACT) / lo (DVE),
           transpose; groups of 4 tiles; within each iq the tile pairs {0,1}
           then {2,3} run 4 resident w2 chunks each, giving late-quantized
           tiles an extra half-iq of runway; w2 chunk loads run on a cursor
           4 chunks ahead of consumption; the next group's quant chains are
           paced proportionally through the group's kc slots (done by slot
           ~21 of 32), with pending transposes force-flushed before any
           matmul that reads them (a read emitted before its write carries
           no dependency).
"""

import sys

sys.path.insert(0, "/opt/trn_rl_repo")

from contextlib import ExitStack

import ml_dtypes
import numpy as np

import concourse.bass as bass
from concourse import bacc
import concourse.mybir as mybir
import concourse.tile as tile
from concourse.alu_op_type import AluOpType as ALU
from concourse.bass_utils import run_bass_kernel_spmd

F32 = mybir.dt.float32
BF16 = mybir.dt.bfloat16
FP8 = mybir.dt.float8e4
I16 = mybir.dt.int16
AXX = mybir.AxisListType.X
GELU = mybir.ActivationFunctionType.Gelu_apprx_tanh
IDENT = mybir.ActivationFunctionType.Identity
DR = mybir.MatmulPerfMode.DoubleRow

B, S, D, H = 4, 4096, 2048, 8192
T = B * S
NCORES = 8
TPC = T // NCORES  # tokens per core
EPS = 1e-5
MAGIC = float(np.float32(1.5 * 2**23))  # add/sub -> round-to-nearest-even
P = 128


CFG = {
    "hqt_bufs": 6,  # tiles 2..15 ring; tiles 0-1 live in hqtA
    "w2_bufs": 6,
    "hst_bufs": 2,
    "hh_bufs": 2,
    "yld_bufs": 4,  # y reload sub-tile ring
    "w1_bufs": 3,
    "yld_nsp": 4,
    "nhq": 4,
    "step_tgt": 21,
    "pair_evac": 0,
    "tail_act_evacs": 0,
    "q01_act_evacs": 1,
    "fill_start": 360,
    "fill_trans": 60,
    "evac_dve": 1,
    "out_bf16": True,
    "pool_mode": "stack",  # tile pool SBUF allocator: stack | queue
    "yld_eng": "scalar",  # engine queue for y16 reloads
    "st_eng": "sync",  # engine queue for y16/out stores
    "tr_eng": "sync",  # engine queue for hqT transposes
}


def build_nc(tpc: int, d: int, h: int) -> bass.Bass:
    NT = tpc // P  # token tiles (16)
    KD = d // P  # layer-1 k-tiles (16)
    KH = h // P  # layer-2 k-tiles (64)
    W1C = 1024  # w1 streamed chunk cols
    NQ = h // W1C  # 8
    TG = 4  # phase-2 token-tile group
    NG = NT // TG
    NI = d // 512  # layer-2 output col chunks (4)
    NKC = CFG.get("NKC", 8)  # w2 k-chunks per iq
    KC = KH // NKC  # k-tiles per w2 chunk
    NHQ = CFG.get("nhq", 4)  # h quantize sub-chunks per tile
    HQC = h // NHQ  # cols per sub-chunk
    KQC = KH // NHQ  # k-tiles per sub-chunk

    nc = bacc.Bacc(trn_type="TRN2")
    x = nc.dram_tensor("x", [tpc, d], F32, kind="ExternalInput")[:]
    w1t = nc.dram_tensor("w1t", [d, h], FP8, kind="ExternalInput")[:]
    w2t = nc.dram_tensor("w2t", [h, d], FP8, kind="ExternalInput")[:]
    wsc = nc.dram_tensor("wsc", [1, 2], F32, kind="ExternalInput")[:]
    out_dt = BF16 if CFG.get("out_bf16") else F32
    out = nc.dram_tensor("out", [tpc, d], out_dt, kind="ExternalOutput")[:]

    def eng(key):
        return getattr(nc, CFG.get(key, "sync"))

    with tile.TileContext(nc, pool_alloc_mode=CFG.get("pool_mode", "stack")) as tc, \
            ExitStack() as ctx:
        const = ctx.enter_context(tc.tile_pool(name="const", bufs=1))
        scl = ctx.enter_context(tc.tile_pool(name="scl", bufs=1))
        dram = ctx.enter_context(tc.tile_pool(name="dram", bufs=1, space="DRAM"))

        wsc_sb = const.tile([P, 2], F32)
        nc.gpsimd.dma_start(out=wsc_sb, in_=wsc.to_broadcast((P, 2)))
        magic_sb = const.tile([P, 1], F32)
        nc.vector.memset(magic_sb, MAGIC)
        nmagic_sb = const.tile([P, 1], F32)
        nc.vector.memset(nmagic_sb, -MAGIC)


        xinv = scl.tile([P, NT], F32)  # (absmax_x/127) * winv1, per tile
        ybuf = dram.tile([tpc, h], I16)
        # running per-tile row max of y, accumulated during phase 1
        ymaxs = [
            scl.tile([P, 1], F32, name=f"ymax{i}", tag="ymax", bufs=NT)
            for i in range(NT)
        ]
        for i in range(NT):
            nc.vector.memset(ymaxs[i], -3.0e38)
        # pools that live across the phase boundary: y reload sub-tiles, the
        # first two h-quant transpose targets, and the small scale tiles, so
        # tiles 0-1's gelu/quant chains can run inside the phase-1 tail
        yld_pool = ctx.enter_context(tc.tile_pool(name="yldq", bufs=CFG["yld_bufs"]))
        hqtA_pool = ctx.enter_context(tc.tile_pool(name="hqtA", bufs=2))
        p2s = ctx.enter_context(tc.tile_pool(name="p2small", bufs=2 * TG))

        ylds: dict = {}  # (tt, j) -> sub tile
        hq_tiles: dict = {}
        pend_tr: list = []
        chp: dict = {}  # chain buffer pools for the current phase

        NSP = CFG.get("yld_nsp", 4)  # y row reloads split into sub-tiles
        YSC = h // NSP

        def ensure_sub(tt, j):
            if (tt, j) in ylds:
                return
            s = yld_pool.tile([P, YSC], I16, tag="yldq", name="yldq")
            eng("yld_eng").dma_start(
                out=s,
                in_=ybuf[tt * P : (tt + 1) * P, j * YSC : (j + 1) * YSC],
            )
            ylds[(tt, j)] = s

        def flush_tr(need=None):
            # transposes go out one step late so their inputs are long since
            # ready and the issuing queue never parks; need=(g, kc) flushes
            # just the entries the imminent matmuls might read
            keep = []
            while pend_tr:
                hh_, tt_, hc_ = pend_tr.pop(0)
                if need is not None:
                    g_, kc_ = need
                    if tt_ >= (g_ + 1) * TG or (
                        tt_ >= g_ * TG and hc_ * KQC > kc_ * KC
                    ):
                        keep.append((hh_, tt_, hc_))
                        continue
                eng("tr_eng").dma_start(
                    out=hq_tiles[tt_][3][:, hc_ * KQC : (hc_ + 1) * KQC, :],
                    in_=hh_.bitcast(BF16).rearrange("p d one -> p (d one)"),
                    transpose=True,
                )
            pend_tr.extend(keep)

        def quant_h_pre(tt):
            # scale chain from the phase-1 running row max of y
            if tt in hq_tiles:
                return
            hm = p2s.tile([P, 1], F32, tag="hm", name="hm")
            nc.scalar.activation(hm, ymaxs[tt], GELU, scale=xinv[:, tt : tt + 1])
            nc.vector.tensor_scalar_max(hm, hm, EPS)
            hs = p2s.tile([P, 1], F32, tag="hs", name="hs")
            nc.vector.reciprocal(hs, hm)
            nc.vector.tensor_scalar(hs, hs, 127.0, None, op0=ALU.mult)
            hinv = p2s.tile([P, 1], F32, tag="hinv", name="hinv")
            nc.vector.tensor_scalar(
                hinv, hm, wsc_sb[:, 1:2], 1.0 / 127.0,
                op0=ALU.mult, op1=ALU.mult,
            )
            if tt < 2:
                hqT16 = hqtA_pool.tile([P, KH, P], BF16, tag="hqTA", name="hqTA")
            else:
                hqT16 = chp["hqt"].tile([P, KH, P], BF16, tag="hqT", name="hqT")
            hq_tiles[tt] = [
                hqT16.bitcast(FP8).rearrange("p k (t two) -> p k t two", two=2),
                hinv,
                hs,
                hqT16,
            ]

        def chain_step(tt, hc, peek=None, hi_pool=False):
            # one NHQ-sub-chunk of the gelu->round->hi/lo->transpose chain
            _, _, hs, hqT16 = hq_tiles[tt]
            ensure_sub(tt, hc * HQC // YSC)
            j = hc * HQC // YSC
            yl = ylds[(tt, j)]
            loc = hc * HQC - j * YSC
            hst_pool, hst_tag = chp["hst"]
            hh_pool, hh_tag = chp["hh"]
            hf = hst_pool.tile([P, HQC], F32, tag=hst_tag, name=hst_tag)
            nc.scalar.activation(
                hf, yl[:, loc : loc + HQC], GELU, scale=xinv[:, tt : tt + 1]
            )
            # round in place: hf <- hf*hs + MAGIC (saves a buffer)
            nc.gpsimd.tensor_scalar(
                hf, hf, hs, MAGIC, op0=ALU.mult, op1=ALU.add
            )
            hh = hh_pool.tile([P, HQC, 2], FP8, tag=hh_tag, name=hh_tag)
            if hi_pool:
                # tail: ACT is loaded with gelu+evacs; cast hi on Pool
                nc.gpsimd.tensor_scalar(
                    hh[:, :, 0], hf, MAGIC, None, op0=ALU.subtract
                )
            else:
                nc.scalar.activation(
                    hh[:, :, 0], hf, IDENT, bias=nmagic_sb[:, 0:1]
                )
            nc.vector.scalar_tensor_tensor(
                hh[:, :, 1], hf, MAGIC, hh[:, :, 0],
                op0=ALU.subtract, op1=ALU.subtract,
            )
            flush_tr()
            pend_tr.append((hh, tt, hc))
            if (hc + 1) * HQC % YSC == 0:
                ylds.pop((tt, j), None)
            if peek is not None and peek[0] in hq_tiles:
                ensure_sub(peek[0], peek[1] * HQC // YSC)

        # ---------- phase 0 + 1: quantize x, y = xq @ w1q -> int16 ----------
        with (
            tc.tile_pool(name="xqt", bufs=1) as xqt_pool,
            tc.tile_pool(name="w1sb", bufs=CFG.get("w1_bufs", 2)) as w1_pool,
            tc.tile_pool(name="xst", bufs=2) as xst,
            tc.tile_pool(name="xqm", bufs=2) as xqm_pool,
            tc.tile_pool(name="xhilo", bufs=2) as xhilo_pool,
            tc.tile_pool(name="p0small", bufs=4) as p0s,
            tc.tile_pool(name="y16", bufs=CFG.get("y16_bufs", 3)) as y16_pool,
            tc.tile_pool(name="mm1", bufs=4, space="PSUM") as mmps,
        ):
            xqT16 = xqt_pool.tile([P, KD, NT * P], BF16)
            xqT8 = xqT16.bitcast(FP8).rearrange("p k (t two) -> p k t two", two=2)

            def quant_x(tt):
                xt = xst.tile([P, d], F32, tag="xt", name="xt")
                if tt < CFG.get("x_split", 0):
                    # startup: halve the load latency by splitting across the
                    # SP and ACT dma queues
                    hd = d // 2
                    nc.sync.dma_start(
                        out=xt[:, :hd], in_=x[tt * P : (tt + 1) * P, :hd]
                    )
                    nc.scalar.dma_start(
                        out=xt[:, hd:], in_=x[tt * P : (tt + 1) * P, hd:]
                    )
                else:
                    nc.sync.dma_start(out=xt, in_=x[tt * P : (tt + 1) * P, :])
                xm = p0s.tile([P, 1], F32, tag="xm", name="xm")
                nc.vector.reduce_max(xm, xt, axis=AXX, apply_absolute_value=True)
                nc.vector.tensor_scalar_max(xm, xm, EPS)
                xs_ = p0s.tile([P, 1], F32, tag="xs", name="xs")
                nc.vector.reciprocal(xs_, xm)
                nc.vector.tensor_scalar(xs_, xs_, 127.0, None, op0=ALU.mult)
                nc.vector.tensor_scalar(
                    xinv[:, tt : tt + 1], xm, wsc_sb[:, 0:1], 1.0 / 127.0,
                    op0=ALU.mult, op1=ALU.mult,
                )
                xq_m = xqm_pool.tile([P, d], F32, tag="xqm", name="xqm")
                nc.scalar.activation(xq_m, xt, IDENT, bias=magic_sb[:, 0:1], scale=xs_)
                hilo = xhilo_pool.tile([P, d, 2], FP8, tag="hilo", name="hilo")
                nc.gpsimd.tensor_scalar(
                    hilo[:, :, 0], xq_m, MAGIC, None, op0=ALU.subtract
                )
                # DVE is the bottleneck engine while the quant pipeline runs
                # (absmax + row-max + evacs); optionally split lo onto Pool
                le = nc.gpsimd if CFG.get("lo_pool") else nc.vector
                le.scalar_tensor_tensor(
                    hilo[:, :, 1], xq_m, MAGIC, hilo[:, :, 0],
                    op0=ALU.subtract, op1=ALU.subtract,
                )
                # first tiles' transposes go via ACT so they don't queue
                # behind the x prefetch loads on SP
                te = nc.scalar if tt < CFG.get("xtr_act", 0) else nc.sync
                te.dma_start(
                    out=xqT16[:, :, tt * P : (tt + 1) * P],
                    in_=hilo.bitcast(BF16).rearrange("p d one -> p (d one)"),
                    transpose=True,
                )

            # keep the PE warm from t~0 so the first real matmuls (at ~23us,
            # after tile 0's quant chain) start at full clock. Source data is
            # tile 15's (not-yet-written, garbage) xqT block -- its transpose
            # lands at ~75us so the WAR ordering is harmless; output goes to
            # a scratch PSUM region reset by the first real start=True.
            # filler source: tile 15's (not-yet-written) xqT block -- its
            # transpose lands at ~75us so the WAR ordering is harmless
            fv = xqT16.bitcast(FP8)
            t15 = NT - 1
            flhs = fv[:, 0:2, t15 * 2 * P : t15 * 2 * P + 128]
            frhs = fv[:, 0:2, t15 * 2 * P : t15 * 2 * P + 256]
            def fill1(n):
                # each burst takes a fresh ring slot so it can never alias a
                # live accumulator
                if not n:
                    return
                dps = mmps.tile([P, W1C], F32, tag="mm", name="mm_fill")
                for _ in range(n):
                    nc.tensor.matmul(
                        dps[:, :256], lhsT=flhs, rhs=frhs, start=True,
                        stop=True, perf_mode=DR, skip_group_check=True,
                    )

            fill1(CFG.get("fill_start", 0))

            for tt in range(4):
                quant_x(tt)

            w1sbs = {}

            def load_w1(q):
                w1sb = w1_pool.tile([P, KD, W1C], FP8, tag="w1sb", name="w1sb")
                for k4 in range(0, KD, 4):
                    nc.sync.dma_start(
                        out=w1sb[:, k4 : k4 + 4, :],
                        in_=w1t[
                            k4 * P : (k4 + 4) * P, q * W1C : (q + 1) * W1C
                        ].rearrange("(kk p) c -> p kk c", p=P),
                    )
                w1sbs[q] = w1sb

            pend_store = []

            def flush_stores():
                while pend_store:
                    y16p, qp, ttp = pend_store.pop(0)
                    eng("st_eng").dma_start(
                        out=ybuf[
                            ttp * P : (ttp + 1) * P, qp * W1C : (qp + 1) * W1C
                        ],
                        in_=y16p,
                    )

            def mm1_block(q, tt):
                w1sb = w1sbs[q]
                ps = mmps.tile([P, W1C], F32, tag="mm", name="mm")
                for kp in range(KD // 2):
                    lhi = xqT8[:, 2 * kp : 2 * kp + 2, tt * P : (tt + 1) * P, 0]
                    llo = xqT8[:, 2 * kp : 2 * kp + 2, tt * P : (tt + 1) * P, 1]
                    for j in range(W1C // 512):
                        rhs = w1sb[:, 2 * kp : 2 * kp + 2, j * 512 : (j + 1) * 512]
                        pj = ps[:, j * 512 : (j + 1) * 512]
                        nc.tensor.matmul(
                            pj, lhsT=lhi, rhs=rhs, start=(kp == 0), stop=False,
                            perf_mode=DR, skip_group_check=True,
                        )
                        nc.tensor.matmul(
                            pj, lhsT=llo, rhs=rhs, start=False,
                            stop=(kp == KD // 2 - 1),
                            perf_mode=DR, skip_group_check=True,
                        )
                y16 = y16_pool.tile([P, W1C], I16, tag="y16", name="y16")
                if q == NQ - 1 and tt >= NT - CFG.get("tail_act_evacs", 2):
                    # the last evacs gate mm2's PSUM reuse: keep them on ACT,
                    # which is free at the very end of the tail, rather than
                    # DVE which still has chain work queued
                    nc.scalar.activation(y16, ps, IDENT)
                elif q < 2 and CFG.get("q01_act_evacs"):
                    # while the quant pipeline runs, DVE is the critical
                    # engine -- keep all evacs on ACT
                    nc.scalar.activation(y16, ps, IDENT)
                elif (q * NT + tt) % 2 == 0:
                    nc.scalar.activation(y16, ps, IDENT)
                else:
                    nc.vector.tensor_scalar(y16, ps, 0.0, None, op0=ALU.add)
                # store is deferred one block so the SP queue never parks on
                # the evacuation's completion
                flush_stores()
                pend_store.append((y16, q, tt))
                cm = p0s.tile([P, 1], F32, tag="cm", name="cm")
                # during the interleaved region DVE is the critical engine
                # (absmax + lo for the quant chain); run the y row-max there
                # on Pool instead
                ce = nc.gpsimd if (q < 2 and CFG.get("cm_pool_q01")) else nc.vector
                ce.reduce_max(cm, y16, axis=AXX)
                ce.tensor_tensor(ymaxs[tt], ymaxs[tt], cm, op=ALU.max)

            # chunks 0 and 1 interleaved per tile: two chunk-blocks of PE work
            # per freshly quantized tile so the PE outruns the quant cadence
            load_w1(0)
            load_w1(1)
            for tt in range(NT):
                if tt + 4 < NT:
                    quant_x(tt + 4)
                if tt == 10:
                    load_w1(2)
                if tt >= 4 and tt % 2 == 0:
                    fill1(CFG.get("fill_q01", 0))
                mm1_block(0, tt)
                mm1_block(1, tt)
            # tiles 0-1's gelu/quant chains run inside the q=NQ-1 tail, using
            # the idle phase-0 quant buffers (same shapes) and hqtA, so the
            # first mm2 group can start right at the phase boundary
            chp["hst"] = (xst, "xt")
            chp["hh"] = (xhilo_pool, "hilo")
            # packed into early tail blocks: the last ~5 blocks stay clean so
            # the final evacs/stores (and the w2 primes queued behind them)
            # aren't pushed past the phase boundary
            tail = {
                1: lambda: (quant_h_pre(0), ensure_sub(0, 0)),
                2: lambda: chain_step(0, 0, peek=(0, 1)),
                4: lambda: chain_step(0, 1, peek=(0, 2)),
                6: lambda: chain_step(0, 2, peek=(0, 3)),
                7: lambda: quant_h_pre(1),
                8: lambda: chain_step(0, 3, peek=(1, 0)),
                10: lambda: chain_step(1, 0, peek=(1, 1)),
                12: lambda: chain_step(1, 1, peek=(1, 2)),
                14: lambda: chain_step(1, 2, peek=(1, 3)),
            }
            for q in range(2, NQ):
                for tt in range(NT):
                    if tt == 2 and q + 1 < NQ:
                        load_w1(q + 1)
                    mm1_block(q, tt)
                    if q == NQ - 1 and tt in tail:
                        tail[tt]()
            flush_stores()

        # ---------- phase 2: gelu, quantize h, out = hq @ w2q ----------
        with (
            tc.tile_pool(name="hqt", bufs=CFG["hqt_bufs"]) as hqt_pool,
            tc.tile_pool(name="w2sb", bufs=CFG["w2_bufs"]) as w2_pool,
            tc.tile_pool(name="hst", bufs=CFG["hst_bufs"]) as hst_pool,
            tc.tile_pool(name="hhilo", bufs=CFG["hh_bufs"]) as hh_pool,
            tc.tile_pool(name="ost", bufs=CFG.get("ost_bufs", 4)) as ost_pool,
            tc.tile_pool(name="mm2", bufs=2 * TG, space="PSUM") as mmps2,
        ):
            chp["hst"] = (hst_pool, "hf")
            chp["hh"] = (hh_pool, "hh")
            chp["hqt"] = hqt_pool
            pend_out = []

            def flush_out():
                while pend_out:
                    otp, ttp, iqp = pend_out.pop(0)
                    eng("st_eng").dma_start(
                        out=out[
                            ttp * P : (ttp + 1) * P, iqp * 512 : (iqp + 1) * 512
                        ],
                        in_=otp,
                    )

            # w2 chunk loads run on a cursor AHEAD of consumption so quant-
            # chain DMA bursts (yld + transposes) never starve the PE of w2
            w2seq = [
                (iq, kc)
                for _g in range(NG)
                for iq in range(NI)
                for kc in range(NKC)
            ]
            w2q: list = []
            w2state = [0]

            def w2_load_one(e=None):
                if w2state[0] >= len(w2seq):
                    return
                iq_, kc_ = w2seq[w2state[0]]
                w2state[0] += 1
                w2sb = w2_pool.tile([P, KC, 512], FP8, tag="w2sb", name="w2sb")
                w2ls = CFG.get("w2_ldsplit", 8)
                for k8 in range(0, KC, w2ls):
                    r0 = (kc_ * KC + k8) * P
                    (e or nc.sync).dma_start(
                        out=w2sb[:, k8 : k8 + w2ls, :],
                        in_=w2t[
                            r0 : r0 + w2ls * P, iq_ * 512 : (iq_ + 1) * 512
                        ].rearrange("(kk p) c -> p kk c", p=P),
                    )
                w2q.append(w2sb)

            # prime w2 BEFORE the transition chains so chunk 0 isn't queued
            # behind their transposes/loads
            W2A = CFG.get("w2_ahead", 4)
            pre_e = getattr(nc, CFG["w2_pre_eng"]) if CFG.get("w2_pre_eng") else None
            for _ in range(W2A):
                w2_load_one(pre_e)

            # PE p-state filler: garbage DoubleRow matmuls on resident hqTA
            # data into a scratch PSUM bank (later reset by a real start=True
            # accumulation). Emitted just before a known PE stall, they keep
            # the tensor engine's ramp warm through the gap so the matmuls
            # after it run at full clock instead of paying the ~3us re-ramp.
            def pe_fill(n, src_tt):
                if not n:
                    return
                fl8 = hq_tiles[src_tt][3].bitcast(FP8).rearrange(
                    "p k t -> p (k t)"
                )
                dps = mmps2.tile([P, 512], F32, tag="mm2", name="ps2_fill")
                lhsT = fl8[:, 0:256].rearrange("p (a b) -> p a b", a=2)
                rhs = fl8[:, 0:1024].rearrange("p (a b) -> p a b", a=2)
                for _ in range(n):
                    nc.tensor.matmul(
                        dps, lhsT=lhsT, rhs=rhs, start=True, stop=True,
                        perf_mode=DR, skip_group_check=True,
                    )

            # transition: only the minimum runs before the first matmuls --
            # tiles 2-3's hc=0 chains (needed by pair23's first kc chunks);
            # the rest interleaves into the g=0 emission stream so the
            # scheduler's coarse engine counters don't gate mm2 on them
            for u in range(2, TG):
                quant_h_pre(u)
            chain_step(1, 3, peek=(2, 0))
            # (2,0)/(3,0) are emitted inside pair01's first kc pass (below)
            # so the first matmuls' queue-tick waits exclude their transposes
            trans_head = [(2, 0), (3, 0)]
            trans_rest = [(2 + (i % 2), 1 + i // 2) for i in range(2 * (NHQ - 1))]

            for g in range(NG):
                # next group's quant chains, paced 2 sub-chunks per 3 kc
                # slots so the last transpose lands before the group ends
                if CFG.get("steps_hc_major"):
                    order = [
                        (u, hc) for hc in range(NHQ) for u in range(TG)
                    ]
                else:
                    order = [
                        (u, hc) for u in range(TG) for hc in range(NHQ)
                    ]
                steps = (
                    [((g + 1) * TG + u, hc) for u, hc in order]
                    if g + 1 < NG
                    else []
                )
                if g == 0:
                    steps = trans_rest + steps
                if g + 1 < NG:
                    for u in range(TG):
                        quant_h_pre((g + 1) * TG + u)
                pe_fill(
                    CFG.get("fill_trans", 0) if g == 0 else CFG.get("fill_grp", 0),
                    g * TG,
                )
                si = 0
                slot = 0
                for iq in range(NI):
                    pss = [
                        mmps2.tile([P, 512], F32, tag="mm2", name=f"ps2_{u}")
                        for u in range(TG)
                    ]

                    def mms(kc, w2sb, u):
                        hqT8 = hq_tiles[g * TG + u][0]
                        for kp in range(KC // 2):
                            kg = kc * KC + 2 * kp
                            lhi = hqT8[:, kg : kg + 2, :, 0]
                            llo = hqT8[:, kg : kg + 2, :, 1]
                            rhs = w2sb[:, 2 * kp : 2 * kp + 2, :]
                            nc.tensor.matmul(
                                pss[u], lhsT=lhi, rhs=rhs,
                                start=(kc == 0 and kp == 0), stop=False,
                                perf_mode=DR, skip_group_check=True,
                            )
                            nc.tensor.matmul(
                                pss[u], lhsT=llo, rhs=rhs, start=False,
                                stop=(kc == NKC - 1 and kp == KC // 2 - 1),
                                perf_mode=DR, skip_group_check=True,
                            )

                    def evac(u):
                        tt = g * TG + u
                        hinv = hq_tiles[tt][1]
                        ot = ost_pool.tile([P, 512], out_dt, tag="ot", name="ot")
                        if u % 2 == 0 and not CFG.get("evac_dve"):
                            nc.scalar.activation(ot, pss[u], IDENT, scale=hinv)
                        else:
                            nc.vector.tensor_scalar(
                                ot, pss[u], hinv, None, op0=ALU.mult
                            )
                        flush_out()
                        pend_out.append((ot, tt, iq))

                    # u-paired: tiles {0,1} run 4 kc chunks, then {2,3} reuse
                    # the same resident w2 chunks -- late-quantized tiles get
                    # an extra half-iq of runway at every group boundary, and
                    # each pair's PSUM banks evacuate as soon as it finishes
                    for kh in range(NKC // 4):
                        chunk4 = [w2q.pop(0) for _ in range(4)]
                        for pi, pair in enumerate(((0, 1), (2, 3))):
                            for i4, w2sb in enumerate(chunk4):
                                # pending transposes a coming matmul reads
                                # MUST be emitted before it (a read emitted
                                # before its write carries no dependency)
                                flush_tr(need=(g, kh * 4 + i4))
                                for u in pair:
                                    mms(kh * 4 + i4, w2sb, u)
                                if g == 0 and iq == 0 and kh == 0 and pi == 0 \
                                        and i4 < len(trans_head):
                                    st = trans_head[i4]
                                    nxt = (
                                        trans_head[i4 + 1]
                                        if i4 + 1 < len(trans_head)
                                        else (steps[0] if steps else None)
                                    )
                                    chain_step(*st, peek=nxt)
                                if pi == 1:
                                    w2_load_one()
                                    # proportional pacing: chain steps spread
                                    # evenly, finishing by slot TGT of 32
                                    TGT = CFG.get("step_tgt", 26)
                                    quota = -(-(slot + 1) * len(steps) // TGT)
                                    while (
                                        si < min(len(steps), quota)
                                        and steps[si][0] in hq_tiles
                                    ):
                                        nxt = (
                                            steps[si + 1]
                                            if si + 1 < len(steps)
                                            else None
                                        )
                                        chain_step(*steps[si], peek=nxt)
                                        si += 1
                                    slot += 1
                            if kh == NKC // 4 - 1 and CFG.get("pair_evac", 1):
                                for u in pair:
                                    evac(u)
                    if not CFG.get("pair_evac", 1):
                        for u in range(TG):
                            evac(u)
                while si < len(steps):
                    quant_h_pre(steps[si][0])
                    chain_step(*steps[si])
                    si += 1
            flush_tr()
            flush_out()
    nc.compile()
    return nc


_wq_cache: dict = {}


def _quant_weight_host(w: np.ndarray):
    """Mirror reference _weight_quant: ternary fp8 values + fp32 inverse scale."""
    import hashlib

    w = np.ascontiguousarray(np.asarray(w, dtype=np.float32))
    key = (w.shape, hashlib.md5(w.view(np.uint8)).hexdigest())
    hit = _wq_cache.get(key)
    if hit is not None:
        return hit
    mean = np.maximum(np.mean(np.abs(w), dtype=np.float32), np.float32(EPS))
    scale = np.float32(1.0) / mean
    tern = np.clip(np.round(w * scale), np.float32(-1.0), np.float32(1.0))
    wT = np.ascontiguousarray(tern.T).astype(ml_dtypes.float8_e4m3)
    winv = np.float32(1.0) / scale
    _wq_cache[key] = (wT, winv)
    return wT, winv


_built: dict = {}


def _get_nc(tpc, d, h):
    key = (tpc, d, h)
    if key not in _built:
        _built[key] = build_nc(*key)
    return _built[key]


def run(inputs, trace=False, shapes=None, ncores=NCORES):
    if shapes is None:
        b, s, d, h = B, S, D, H
    else:
        b, s, d, h = shapes
    t = b * s
    tpc = t // ncores
    x = np.ascontiguousarray(np.asarray(inputs["x"], np.float32).reshape(t, d))
    w1t, winv1 = _quant_weight_host(inputs["w1"])
    w2t, winv2 = _quant_weight_host(inputs["w2"])
    wsc = np.array([[winv1, winv2]], dtype=np.float32)
    in_maps = [
        {
            "x": np.ascontiguousarray(x[c * tpc : (c + 1) * tpc]),
            "w1t": w1t,
            "w2t": w2t,
            "wsc": wsc,
        }
        for c in range(ncores)
    ]
    nc = _get_nc(tpc, d, h)
    res = run_bass_kernel_spmd(
        nc, in_maps, core_ids=list(range(ncores)), trace=False
    )
    outf = np.concatenate(
        [np.asarray(res.results[c]["out"], dtype=np.float32) for c in range(ncores)],
        axis=0,
    )
    return outf.reshape(b, s, d), res


def kernel(**inputs) -> np.ndarray:
    return run(inputs)[0]

